# revision 8
# baseline (speedup 1.0000x reference)
"""Trainium2 Bass kernel for nn_MultiHeadLinearAttention.

Full-input contract: kernel(**inputs) takes the unsharded numpy inputs and
returns the full output. Internally: data-parallel over batch across the 8
NeuronCores (B == 8, one batch element per core), no collectives.

Per-core math (S=2048, E=2048, H=16, d=128):
  q/k projections: fp8(e4m3) DoubleRow matmuls, contraction d=128 packed as
    two 64-partition K-tiles (x and W pre-packed [64, 2, *] on host, scales
    x*16, W*256 so the psum holds q*2^12). A tiny fp8 "ones" matmul preloads
    the psum with 2^12 so it holds (q+1)*2^12, which lets
    phi(x) = elu(x)+1 = min(exp(x), max(x+1, 1)) be computed with one Act op
    (Exp with scale 2^-12, bias -1) + one DVE tensor_scalar
    ((in max 4096) * 2^-12) + one DVE min.
  Gram trick (bf16): G[d',d] = sum_s xh[s,d'] pk[s,d]; kv = G^T @ Wv; ksum
    fused as psum col 128.  num/den: bf16; den for all 16 heads of an
    s-chunk lands in one [128,16] psum tile -> single reciprocal.
  ctx: stored as fp8 hi+lo residual pair (scale 2^10), byte-interleaved
    [s, h, d, 2] so ONE uint16-view blocked DMA transpose yields
    [d, h, s, 2] with hi/lo selectable by byte offset.
  Wo stage: fp8 DoubleRow, 3 instructions per head-pair per e-block:
    (c_hi+c_lo) @ w_hi (2 instrs) + c_hi @ w_lo (1 instr) -- 0.75 PE
    cycles per K=128 column vs 1.0 for bf16, with ~bf16-level accuracy
    (Wo pre-split into hi+lo e4m3 on host, scale 2^10).
  Output: psum * 2^-20 -> bf16 SBUF -> DRAM; host upcasts and adds bias.
"""

import numpy as np
import ml_dtypes

import concourse.bass as bass
import concourse.mybir as mybir
import concourse.tile as tile
from concourse import bacc
from concourse.bass_utils import run_bass_kernel_spmd

S = 2048
E = 2048
H = 16
D = 128
N_CORES = 8
NCH = S // 128  # 16 s-chunks
NPAIR = H // 2  # 8 head pairs

F32 = mybir.dt.float32
BF16 = mybir.dt.bfloat16
FP8 = mybir.dt.float8e4
U16 = mybir.dt.uint16
AF = mybir.ActivationFunctionType
ALU = mybir.AluOpType
PM = mybir.MatmulPerfMode

E4NP = ml_dtypes.float8_e4m3
BFNP = ml_dtypes.bfloat16

SX = 16.0       # x -> fp8 scale
SWQ = 256.0     # Wq/Wk -> fp8 scale (psum = q * 2^12)
SC = 1024.0     # ctx -> fp8 scale
SWO = 1024.0    # Wo -> fp8 scale
OUT_DESCALE = 1.0 / (SC * SWO)

_CACHED = {}
DEBUG = False


def build_module():
    nc = bacc.Bacc("TRN2", target_bir_lowering=False, debug=False,
                   num_devices=N_CORES)

    x8 = nc.dram_tensor("x8", [64, H, 2, S], FP8, kind="ExternalInput")
    xn = nc.dram_tensor("xn", [H, 128, NCH * 128], BF16,
                        kind="ExternalInput")
    w8 = nc.dram_tensor("w8", [64, H, 2, 256], FP8, kind="ExternalInput")
    wv = nc.dram_tensor("wv", [128, H * 128], BF16, kind="ExternalInput")
    wo_hi = nc.dram_tensor("wo_hi", [128, NPAIR, 2, E], FP8,
                           kind="ExternalInput")
    wo_lo = nc.dram_tensor("wo_lo", [128, NPAIR, 2, E], FP8,
                           kind="ExternalInput")
    out = nc.dram_tensor("out", [S, E], BF16, kind="ExternalOutput")
    if DEBUG:
        dbg_pqT = nc.dram_tensor("dbg_pqT", [128, H * S], BF16,
                                 kind="ExternalOutput")
        dbg_kv = nc.dram_tensor("dbg_kv", [128, H, 128], BF16,
                                kind="ExternalOutput")
        dbg_ksd = nc.dram_tensor("dbg_ksd", [128, H], BF16,
                                 kind="ExternalOutput")
        dbg_inv = nc.dram_tensor("dbg_inv", [128, 16], F32,
                                 kind="ExternalOutput")
        dbg_chl = nc.dram_tensor("dbg_chl", [128, H, 128, 2], FP8,
                                 kind="ExternalOutput")
        dbg_chlT = nc.dram_tensor("dbg_chlT", [128, H, 128, 2], FP8,
                                  kind="ExternalOutput")
        dbg_g = nc.dram_tensor("dbg_g", [128, 128], BF16,
                               kind="ExternalOutput")

    with tile.TileContext(nc) as tc:
        with (
            tc.tile_pool(name="const", bufs=1) as const,
            tc.tile_pool(name="work", bufs=2) as work,
            tc.tile_pool(name="psum", bufs=2, space="PSUM") as psum,
        ):
            # ---------------- constants / persistent tiles ----------------
            wv_sb = const.tile([128, H * 128], BF16)
            nc.sync.dma_start(out=wv_sb[:], in_=wv[:])
            woh_sb = const.tile([128, NPAIR, 2, E], FP8)
            nc.sync.dma_start(out=woh_sb[:], in_=wo_hi[:])
            wol_sb = const.tile([128, NPAIR, 2, E], FP8)
            nc.sync.dma_start(out=wol_sb[:], in_=wo_lo[:])
            w8_sb = const.tile([64, H, 2, 256], FP8)
            nc.sync.dma_start(out=w8_sb[:], in_=w8[:])

            pre_l = const.tile([1, 2, 128], FP8)
            nc.vector.memset(pre_l[:], 32.0)
            pre_r = const.tile([1, 2, 512], FP8)
            nc.vector.memset(pre_r[:], 64.0)
            neg1 = const.tile([128, 1], F32)
            nc.vector.memset(neg1[:], -1.0)
            ones = const.tile([128, 1], BF16)
            nc.vector.memset(ones[:], 1.0)

            pqT = const.tile([128, H * S], BF16)       # all heads, transposed
            kv_all = const.tile([128, H, 128], BF16)   # kv per head
            ksd = const.tile([128, H], BF16)           # ksum * 2^-10 per head

            # ---------------- helpers ----------------
            def phi(src_psum, dst, n):
                """psum holds (v+1)*2^12; writes phi(v) [128, n] bf16."""
                e = work.tile([128, 512], BF16, tag="e", bufs=2)
                nc.scalar.activation(e[:, 0:n], src_psum[:, 0:n], AF.Exp,
                                     bias=neg1[:, 0:1], scale=2.0 ** -12)
                t = work.tile([128, 512], BF16, tag="t", bufs=2)
                nc.vector.tensor_scalar(t[:, 0:n], src_psum[:, 0:n], 4096.0,
                                        2.0 ** -12, ALU.max, ALU.mult)
                nc.vector.tensor_tensor(dst, e[:, 0:n], t[:, 0:n], ALU.min)

            def preload(p, n):
                nc.tensor.matmul(p[:, 0:n], pre_l[:], pre_r[:, :, 0:n],
                                 start=True, stop=False,
                                 perf_mode=PM.DoubleRow)

            # deferred q quarters: (h, quarter) for quarters 1..3
            q_queue = [(h, qt) for qt in (1, 2, 3) for h in range(H)]
            q_queue.reverse()
            q_pf = list(q_queue)
            q_pending = []

            def prefetch_q_quarter():
                h, qt = q_pf.pop()
                xq = work.tile([64, 2, 512], FP8, tag="xq", bufs=3)
                nc.sync.dma_start(
                    out=xq[:], in_=x8[:, h, :, qt * 512:(qt + 1) * 512])
                q_pending.append(xq)

            def emit_q_quarter(h, qt):
                xq = q_pending.pop(0)
                qp = psum.tile([128, 512], F32, tag="qk", bufs=2)
                preload(qp, 512)
                nc.tensor.matmul(
                    qp[:], w8_sb[:, h, :, 0:128], xq[:],
                    start=False, stop=True, perf_mode=PM.DoubleRow)
                phi(qp, pqT[:, h * S + qt * 512:h * S + (qt + 1) * 512], 512)

            def drain_q(k):
                for _ in range(k):
                    while q_pf and len(q_pending) < 2:
                        prefetch_q_quarter()
                    if q_queue:
                        emit_q_quarter(*q_queue.pop())

            # ---------------- pass A: per head ----------------
            for h in range(H):
                x8_h = work.tile([64, 2, S], FP8, tag="x8h", bufs=3)
                nc.sync.dma_start(out=x8_h[:], in_=x8[:, h])
                xn_h = work.tile([128, NCH, 128], BF16, tag="xn", bufs=2)
                nc.sync.dma_start(
                    out=xn_h[:], in_=xn[h].rearrange("p (c j) -> p c j",
                                                     j=128))

                # k projection (natural layout) + phi -> pk
                pk = work.tile([128, NCH, 128], BF16, tag="pk", bufs=2)
                for g in range(4):
                    kp = psum.tile([128, 512], F32, tag="qk", bufs=2)
                    preload(kp, 512)
                    for c in range(4):
                        ch = g * 4 + c
                        nc.tensor.matmul(
                            kp[:, c * 128:(c + 1) * 128],
                            x8_h[:, :, ch * 128:(ch + 1) * 128],
                            w8_sb[:, h, :, 128:256],
                            start=False, stop=(c == 3),
                            perf_mode=PM.DoubleRow)
                    phi(kp, pk[:, 4 * g:4 * g + 4, :].rearrange(
                        "p c j -> p (c j)"), 512)

                # q projection quarter 0 (transposed layout)
                qp = psum.tile([128, 512], F32, tag="qk", bufs=2)
                preload(qp, 512)
                nc.tensor.matmul(qp[:], w8_sb[:, h, :, 0:128],
                                 x8_h[:, :, 0:512], start=False, stop=True,
                                 perf_mode=PM.DoubleRow)
                phi(qp, pqT[:, h * S:h * S + 512], 512)

                # Gram G + ksum (bf16)
                gp_t = psum.tile([128, 128], F32, tag="g", bufs=1)
                gp = gp_t[:]
                kvp_t = psum.tile([128, 129], F32, tag="kv", bufs=1)
                kvp = kvp_t[:]
                for c in range(NCH):
                    nc.tensor.matmul(gp, xn_h[:, c, :], pk[:, c, :],
                                     start=(c == 0), stop=(c == NCH - 1),
                                     skip_group_check=True)
                    nc.tensor.matmul(kvp[:, 128:129], pk[:, c, :], ones[:],
                                     start=(c == 0), stop=(c == NCH - 1),
                                     skip_group_check=True)
                g_sb = work.tile([128, 128], BF16, tag="gsb", bufs=2)
                nc.scalar.activation(g_sb[:], gp, AF.Copy)
                if DEBUG and h == 0:
                    nc.sync.dma_start(out=dbg_g[:], in_=g_sb[:])
                nc.tensor.matmul(kvp[:, 0:128], g_sb[:],
                                 wv_sb[:, h * 128:(h + 1) * 128],
                                 start=True, stop=True, skip_group_check=True)
                nc.scalar.activation(kv_all[:, h, :], kvp[:, 0:128], AF.Copy)
                nc.scalar.activation(ksd[:, h:h + 1], kvp[:, 128:129],
                                     AF.Copy, scale=2.0 ** -10)

            if DEBUG:
                nc.sync.dma_start(out=dbg_pqT[:, 0:512], in_=pqT[:, 0:512])
                nc.sync.dma_start(out=dbg_kv[:], in_=kv_all[:])
                nc.sync.dma_start(out=dbg_ksd[:], in_=ksd[:])
            # ---------------- pass B: per s-chunk ----------------
            for sc in range(NCH):
                # den for all heads -> one [128, 16] psum; then reciprocal
                den_t = psum.tile([128, 129], F32, tag="kv", bufs=1)
                denp = den_t[:, 0:16]
                for h in range(H):
                    nc.tensor.matmul(
                        denp[:, h:h + 1],
                        pqT[:, h * S + sc * 128:h * S + (sc + 1) * 128],
                        ksd[:, h:h + 1], start=True, stop=True,
                        skip_group_check=True)
                invsc = work.tile([128, 16], F32, tag="inv", bufs=2)
                nc.vector.reciprocal(invsc[:], denp)
                if DEBUG and sc == 0:
                    nc.sync.dma_start(out=dbg_inv[:], in_=invsc[:])  # = 1024/den

                drain_q(1)

                # num + ctx hi/lo (byte-interleaved) per head
                chl = work.tile([128, H, 128, 2], FP8, tag="chl", bufs=2)
                for h in range(H):
                    ndp = psum.tile([128, 128], F32, tag="nd", bufs=2)
                    nc.tensor.matmul(
                        ndp[:],
                        pqT[:, h * S + sc * 128:h * S + (sc + 1) * 128],
                        kv_all[:, h, :], start=True, stop=True)
                    nc.vector.tensor_scalar(chl[:, h, :, 0], ndp[:],
                                            invsc[:, h:h + 1], None, ALU.mult)
                    nc.vector.scalar_tensor_tensor(
                        chl[:, h, :, 1], ndp[:], invsc[:, h:h + 1],
                        chl[:, h, :, 0], ALU.mult, ALU.subtract)
                    if h in (5, 11):
                        drain_q(1)

                # one blocked u16 transpose: [s,(h d)] -> [d, h, s] pairs
                if DEBUG and sc == 0:
                    nc.sync.dma_start(out=dbg_chl[:], in_=chl[:])
                chlT = work.tile([128, H, 128, 2], FP8, tag="chlT", bufs=2)
                nc.sync.dma_start(
                    out=chlT[:].rearrange("p h s two -> p h (s two)").bitcast(
                        U16),
                    in_=chl[:].rearrange("p h d two -> p (h d two)").bitcast(
                        U16),
                    transpose=True)

                if DEBUG and sc == 0:
                    nc.sync.dma_start(out=dbg_chlT[:], in_=chlT[:])
                # Wo stage: fp8 DR, 3 instrs per head-pair per e-block
                for eb in range(4):
                    wop = psum.tile([128, 512], F32, tag="wo", bufs=2)
                    esl = slice(eb * 512, (eb + 1) * 512)
                    for j in range(NPAIR):
                        hsl = slice(2 * j, 2 * j + 2)
                        nc.tensor.matmul(
                            wop[:], chlT[:, hsl, :, 0], woh_sb[:, j, :, esl],
                            start=(j == 0), stop=False,
                            perf_mode=PM.DoubleRow)
                        nc.tensor.matmul(
                            wop[:], chlT[:, hsl, :, 1], woh_sb[:, j, :, esl],
                            start=False, stop=False, perf_mode=PM.DoubleRow)
                        nc.tensor.matmul(
                            wop[:], chlT[:, hsl, :, 0], wol_sb[:, j, :, esl],
                            start=False, stop=(j == NPAIR - 1),
                            perf_mode=PM.DoubleRow)
                    out_sb = work.tile([128, 512], BF16, tag="osb", bufs=3)
                    nc.scalar.activation(out_sb[:], wop[:], AF.Copy,
                                         scale=OUT_DESCALE)
                    nc.sync.dma_start(
                        out=out[sc * 128:(sc + 1) * 128, esl], in_=out_sb[:])
                    drain_q(1)

            if DEBUG:
                nc.sync.dma_start(out=dbg_pqT[:], in_=pqT[:])
    nc.compile()
    return nc


def get_module():
    if "nc" not in _CACHED:
        _CACHED["nc"] = build_module()
    return _CACHED["nc"]


def _e4(a, scale):
    return (np.ascontiguousarray(a, dtype=np.float32) * scale).astype(E4NP)


def _bf(a):
    return np.ascontiguousarray(a, dtype=np.float32).astype(BFNP)


def prepare_in_maps(inputs, Wq, Wk, Wv, Wo, bo):
    """Host-side shard + layout prep. Returns per-core input maps."""
    Wq = np.asarray(Wq, dtype=np.float32)
    Wk = np.asarray(Wk, dtype=np.float32)
    Wv = np.asarray(Wv, dtype=np.float32)
    Wo = np.asarray(Wo, dtype=np.float32)
    # w8: [64, H, 2, 256] = (Wq | Wk) * 256
    wq_p = np.transpose(Wq.reshape(H, 2, 64, D), (2, 0, 1, 3))
    wk_p = np.transpose(Wk.reshape(H, 2, 64, D), (2, 0, 1, 3))
    w8_p = _e4(np.concatenate([wq_p, wk_p], axis=3), SWQ)
    # wv: [128, H*128]
    wv_p = _bf(np.transpose(Wv, (1, 0, 2)).reshape(D, H * D))
    # wo hi/lo: [128, NPAIR, 2, E] with residual split at scale SWO
    wo_r = np.transpose(Wo.reshape(NPAIR, 2, D, E), (2, 0, 1, 3))
    wo_hi = _e4(wo_r, SWO)
    wo_lo = (wo_r * SWO - wo_hi.astype(np.float32)).astype(E4NP)
    in_maps = []
    for b in range(N_CORES):
        xb = np.asarray(inputs[b], dtype=np.float32)
        # x8: [64, H, 2, S]: x8[p,h,i,s] = 16*x[s, h*128+i*64+p]
        x8_p = _e4(np.transpose(xb.reshape(S, H, 2, 64), (3, 1, 2, 0)), SX)
        # xn packed per head: xn[h][p, c*128+j] = x[c*128+p, h*128+j]
        xn_p = _bf(np.transpose(xb.reshape(NCH, 128, H, D),
                                (2, 1, 0, 3)).reshape(H, 128, NCH * D))
        in_maps.append({"x8": x8_p, "xn": xn_p, "w8": w8_p, "wv": wv_p,
                        "wo_hi": wo_hi, "wo_lo": wo_lo})
    return in_maps


def kernel(inputs, Wq, Wk, Wv, Wo, bo):
    B = inputs.shape[0]
    assert B == N_CORES and inputs.shape[1:] == (S, E)
    nc = get_module()
    in_maps = prepare_in_maps(inputs, Wq, Wk, Wv, Wo, bo)
    res = run_bass_kernel_spmd(nc, in_maps, list(range(N_CORES)))
    outs = np.stack([res.results[b]["out"].astype(np.float32)
                     for b in range(N_CORES)], axis=0)
    return (outs + np.asarray(bo, dtype=np.float32)[None, None, :]).astype(
        np.float32)


# revision 10
# speedup vs baseline: 1.0504x; 1.0504x over previous
"""Trainium2 Bass kernel for nn_MultiHeadLinearAttention.

Full-input contract: kernel(**inputs) takes the unsharded numpy inputs and
returns the full output. Internally: data-parallel over batch across the 8
NeuronCores (B == 8, one batch element per core), no collectives.

Per-core math (S=2048, E=2048, H=16, d=128):
  q/k projections: fp8(e4m3) DoubleRow matmuls, contraction d=128 packed as
    two 64-partition K-tiles (x and W pre-packed [64, 2, *] on host, scales
    x*16, W*256 so the psum holds q*2^12). A tiny fp8 "ones" matmul preloads
    the psum with 2^12 so it holds (q+1)*2^12, which lets
    phi(x) = elu(x)+1 = min(exp(x), max(x+1, 1)) be computed with one Act op
    (Exp with scale 2^-12, bias -1) + one DVE tensor_scalar
    ((in max 4096) * 2^-12) + one DVE min.
  Gram trick (bf16): G[d',d] = sum_s xh[s,d'] pk[s,d]; kv = G^T @ Wv; ksum
    fused as psum col 128.  num/den: bf16; den for all 16 heads of an
    s-chunk lands in one [128,16] psum tile -> single reciprocal.
  ctx: stored as fp8 hi+lo residual pair (scale 2^10), byte-interleaved
    [s, h, d, 2] so ONE uint16-view blocked DMA transpose yields
    [d, h, s, 2] with hi/lo selectable by byte offset.
  Wo stage: fp8 DoubleRow, 3 instructions per head-pair per e-block:
    (c_hi+c_lo) @ w_hi (2 instrs) + c_hi @ w_lo (1 instr) -- 0.75 PE
    cycles per K=128 column vs 1.0 for bf16, with ~bf16-level accuracy
    (Wo pre-split into hi+lo e4m3 on host, scale 2^10).
  Output: psum * 2^-20 -> bf16 SBUF -> DRAM; host upcasts and adds bias.
"""

import numpy as np
import ml_dtypes

import concourse.bass as bass
import concourse.mybir as mybir
import concourse.tile as tile
from concourse import bacc
from concourse.bass_utils import run_bass_kernel_spmd

S = 2048
E = 2048
H = 16
D = 128
N_CORES = 8
NCH = S // 128  # 16 s-chunks
NPAIR = H // 2  # 8 head pairs

F32 = mybir.dt.float32
BF16 = mybir.dt.bfloat16
FP8 = mybir.dt.float8e4
U16 = mybir.dt.uint16
AF = mybir.ActivationFunctionType
ALU = mybir.AluOpType
PM = mybir.MatmulPerfMode

E4NP = ml_dtypes.float8_e4m3
BFNP = ml_dtypes.bfloat16

SX = 16.0       # x -> fp8 scale
SWQ = 256.0     # Wq/Wk -> fp8 scale (psum = q * 2^12)
SC = 1024.0     # ctx -> fp8 scale
SWO = 1024.0    # Wo -> fp8 scale
OUT_DESCALE = 1.0 / (SC * SWO)

_CACHED = {}
DEBUG = False


def build_module():
    nc = bacc.Bacc("TRN2", target_bir_lowering=False, debug=False,
                   num_devices=N_CORES)

    x8 = nc.dram_tensor("x8", [64, H, 2, S], FP8, kind="ExternalInput")
    xn = nc.dram_tensor("xn", [H, 128, NCH * 128], BF16,
                        kind="ExternalInput")
    w8 = nc.dram_tensor("w8", [64, H, 2, 256], FP8, kind="ExternalInput")
    wv = nc.dram_tensor("wv", [128, H * 128], BF16, kind="ExternalInput")
    wo_hi = nc.dram_tensor("wo_hi", [128, NPAIR, 2, E], FP8,
                           kind="ExternalInput")
    wo_lo = nc.dram_tensor("wo_lo", [128, NPAIR, 2, E], FP8,
                           kind="ExternalInput")
    out = nc.dram_tensor("out", [S, E], BF16, kind="ExternalOutput")
    if DEBUG:
        dbg_pqT = nc.dram_tensor("dbg_pqT", [128, H * S], BF16,
                                 kind="ExternalOutput")
        dbg_kv = nc.dram_tensor("dbg_kv", [128, H, 128], BF16,
                                kind="ExternalOutput")
        dbg_ksd = nc.dram_tensor("dbg_ksd", [128, H], BF16,
                                 kind="ExternalOutput")
        dbg_inv = nc.dram_tensor("dbg_inv", [128, 16], F32,
                                 kind="ExternalOutput")
        dbg_chl = nc.dram_tensor("dbg_chl", [128, H, 128, 2], FP8,
                                 kind="ExternalOutput")
        dbg_chlT = nc.dram_tensor("dbg_chlT", [128, H, 128, 2], FP8,
                                  kind="ExternalOutput")
        dbg_g = nc.dram_tensor("dbg_g", [128, 128], BF16,
                               kind="ExternalOutput")

    with tile.TileContext(nc) as tc:
        with (
            tc.tile_pool(name="const", bufs=1) as const,
            tc.tile_pool(name="work", bufs=2) as work,
            tc.tile_pool(name="psum", bufs=2, space="PSUM") as psum,
        ):
            # ---------------- constants / persistent tiles ----------------
            wv_sb = const.tile([128, H * 128], BF16)
            nc.sync.dma_start(out=wv_sb[:], in_=wv[:])
            woh_sb = const.tile([128, NPAIR, 2, E], FP8)
            nc.sync.dma_start(out=woh_sb[:], in_=wo_hi[:])
            wol_sb = const.tile([128, NPAIR, 2, E], FP8)
            nc.sync.dma_start(out=wol_sb[:], in_=wo_lo[:])
            w8_sb = const.tile([64, H, 2, 256], FP8)
            nc.sync.dma_start(out=w8_sb[:], in_=w8[:])

            pre_l = const.tile([1, 2, 128], FP8)
            nc.vector.memset(pre_l[:], 32.0)
            pre_r = const.tile([1, 2, 512], FP8)
            nc.vector.memset(pre_r[:], 64.0)
            neg1 = const.tile([128, 1], F32)
            nc.vector.memset(neg1[:], -1.0)
            ones = const.tile([128, 1], BF16)
            nc.vector.memset(ones[:], 1.0)

            pqT = const.tile([128, H * S], BF16)       # all heads, transposed
            kv_all = const.tile([128, H, 128], BF16)   # kv per head
            ksd = const.tile([128, H], BF16)           # ksum * 2^-10 per head

            # ---------------- helpers ----------------
            def phi(src_psum, dst, n):
                """psum holds (v+1)*2^12; writes phi(v) [128, n] bf16."""
                e = work.tile([128, 512], BF16, tag="e", bufs=2)
                nc.scalar.activation(e[:, 0:n], src_psum[:, 0:n], AF.Exp,
                                     bias=neg1[:, 0:1], scale=2.0 ** -12)
                t = work.tile([128, 512], BF16, tag="t", bufs=2)
                nc.vector.tensor_scalar(t[:, 0:n], src_psum[:, 0:n], 4096.0,
                                        2.0 ** -12, ALU.max, ALU.mult)
                nc.vector.tensor_tensor(dst, e[:, 0:n], t[:, 0:n], ALU.min)

            def preload(p, n):
                nc.tensor.matmul(p[:, 0:n], pre_l[:], pre_r[:, :, 0:n],
                                 start=True, stop=False,
                                 perf_mode=PM.DoubleRow)

            # deferred q quarters: (h, quarter) for quarters 1..3
            q_queue = [(h, qt) for qt in (1, 2, 3) for h in range(H)]
            q_queue.reverse()
            q_pf = list(q_queue)
            q_pending = []

            def prefetch_q_quarter():
                h, qt = q_pf.pop()
                xq = work.tile([64, 2, 512], FP8, tag="xq", bufs=3)
                nc.sync.dma_start(
                    out=xq[:], in_=x8[:, h, :, qt * 512:(qt + 1) * 512])
                q_pending.append(xq)

            def emit_q_quarter(h, qt):
                xq = q_pending.pop(0)
                qp = psum.tile([128, 512], F32, tag="qk", bufs=2)
                preload(qp, 512)
                nc.tensor.matmul(
                    qp[:], w8_sb[:, h, :, 0:128], xq[:],
                    start=False, stop=True, perf_mode=PM.DoubleRow)
                phi(qp, pqT[:, h * S + qt * 512:h * S + (qt + 1) * 512], 512)

            def drain_q(k):
                for _ in range(k):
                    while q_pf and len(q_pending) < 2:
                        prefetch_q_quarter()
                    if q_queue:
                        emit_q_quarter(*q_queue.pop())

            # ---------------- pass A: per head ----------------
            for h in range(H):
                x8_h = work.tile([64, 2, S], FP8, tag="x8h", bufs=2)
                nc.sync.dma_start(out=x8_h[:], in_=x8[:, h])
                xn_h = work.tile([128, NCH, 128], BF16, tag="xn", bufs=2)
                nc.sync.dma_start(
                    out=xn_h[:], in_=xn[h].rearrange("p (c j) -> p c j",
                                                     j=128))

                # k projection (natural layout) + phi -> pk
                pk = work.tile([128, NCH, 128], BF16, tag="pk", bufs=2)
                for g in range(4):
                    kp = psum.tile([128, 512], F32, tag="qk", bufs=2)
                    preload(kp, 512)
                    for c in range(4):
                        ch = g * 4 + c
                        nc.tensor.matmul(
                            kp[:, c * 128:(c + 1) * 128],
                            x8_h[:, :, ch * 128:(ch + 1) * 128],
                            w8_sb[:, h, :, 128:256],
                            start=False, stop=(c == 3),
                            perf_mode=PM.DoubleRow)
                    phi(kp, pk[:, 4 * g:4 * g + 4, :].rearrange(
                        "p c j -> p (c j)"), 512)

                # q projection quarter 0 (transposed layout)
                qp = psum.tile([128, 512], F32, tag="qk", bufs=2)
                preload(qp, 512)
                nc.tensor.matmul(qp[:], w8_sb[:, h, :, 0:128],
                                 x8_h[:, :, 0:512], start=False, stop=True,
                                 perf_mode=PM.DoubleRow)
                phi(qp, pqT[:, h * S:h * S + 512], 512)

                # Gram G + ksum (bf16)
                gp_t = psum.tile([128, 128], F32, tag="g", bufs=1)
                gp = gp_t[:]
                kvp_t = psum.tile([128, 129], F32, tag="kv", bufs=1)
                kvp = kvp_t[:]
                for c in range(NCH):
                    nc.tensor.matmul(gp, xn_h[:, c, :], pk[:, c, :],
                                     start=(c == 0), stop=(c == NCH - 1),
                                     skip_group_check=True)
                    nc.tensor.matmul(kvp[:, 128:129], pk[:, c, :], ones[:],
                                     start=(c == 0), stop=(c == NCH - 1),
                                     skip_group_check=True)
                g_sb = work.tile([128, 128], BF16, tag="gsb", bufs=2)
                nc.scalar.activation(g_sb[:], gp, AF.Copy)
                if DEBUG and h == 0:
                    nc.sync.dma_start(out=dbg_g[:], in_=g_sb[:])
                nc.tensor.matmul(kvp[:, 0:128], g_sb[:],
                                 wv_sb[:, h * 128:(h + 1) * 128],
                                 start=True, stop=True, skip_group_check=True)
                nc.scalar.activation(kv_all[:, h, :], kvp[:, 0:128], AF.Copy)
                nc.scalar.activation(ksd[:, h:h + 1], kvp[:, 128:129],
                                     AF.Copy, scale=2.0 ** -10)

            if DEBUG:
                nc.sync.dma_start(out=dbg_pqT[:, 0:512], in_=pqT[:, 0:512])
                nc.sync.dma_start(out=dbg_kv[:], in_=kv_all[:])
                nc.sync.dma_start(out=dbg_ksd[:], in_=ksd[:])
            # ---------------- pass B: per s-chunk ----------------
            for sc in range(NCH):
                # den for all heads -> one [128, 16] psum; then reciprocal
                den_t = psum.tile([128, 129], F32, tag="kv", bufs=1)
                denp = den_t[:, 0:16]
                for h in range(H):
                    nc.tensor.matmul(
                        denp[:, h:h + 1],
                        pqT[:, h * S + sc * 128:h * S + (sc + 1) * 128],
                        ksd[:, h:h + 1], start=True, stop=True,
                        skip_group_check=True)
                invsc = work.tile([128, 16], F32, tag="inv", bufs=2)
                nc.vector.reciprocal(invsc[:], denp)
                if DEBUG and sc == 0:
                    nc.sync.dma_start(out=dbg_inv[:], in_=invsc[:])  # = 1024/den

                drain_q(1)

                # num + ctx scale to bf16 per head (DVE/Act alternating),
                # then one batched fp8 hi-cast + one batched lo-subtract
                chl = work.tile([128, H, 128, 2], FP8, tag="chl", bufs=2)
                cbf = work.tile([128, H, 128], BF16, tag="cbf", bufs=2)
                for h in range(H):
                    ndp = psum.tile([128, 128], F32, tag="nd", bufs=2)
                    nc.tensor.matmul(
                        ndp[:],
                        pqT[:, h * S + sc * 128:h * S + (sc + 1) * 128],
                        kv_all[:, h, :], start=True, stop=True)
                    if h % 2 == 0:
                        nc.vector.tensor_scalar(cbf[:, h, :], ndp[:],
                                                invsc[:, h:h + 1], None,
                                                ALU.mult)
                    else:
                        nc.scalar.activation(cbf[:, h, :], ndp[:], AF.Copy,
                                             scale=invsc[:, h:h + 1])
                    if h in (5, 11):
                        drain_q(1)
                nc.scalar.activation(chl[:, :, :, 0], cbf[:], AF.Copy)
                nc.vector.tensor_tensor(chl[:, :, :, 1], cbf[:],
                                        chl[:, :, :, 0], ALU.subtract)

                # one blocked u16 transpose: [s,(h d)] -> [d, h, s] pairs
                if DEBUG and sc == 0:
                    nc.sync.dma_start(out=dbg_chl[:], in_=chl[:])
                chlT = work.tile([128, H, 128, 2], FP8, tag="chlT", bufs=2)
                nc.scalar.dma_start(
                    out=chlT[:].rearrange("p h s two -> p h (s two)").bitcast(
                        U16),
                    in_=chl[:].rearrange("p h d two -> p (h d two)").bitcast(
                        U16),
                    transpose=True)

                if DEBUG and sc == 0:
                    nc.sync.dma_start(out=dbg_chlT[:], in_=chlT[:])
                # Wo stage: fp8 DR, 3 instrs per head-pair per e-block
                for eb in range(4):
                    wop = psum.tile([128, 512], F32, tag="wo", bufs=2)
                    esl = slice(eb * 512, (eb + 1) * 512)
                    for j in range(NPAIR):
                        hsl = slice(2 * j, 2 * j + 2)
                        nc.tensor.matmul(
                            wop[:], chlT[:, hsl, :, 0], woh_sb[:, j, :, esl],
                            start=(j == 0), stop=False,
                            perf_mode=PM.DoubleRow)
                        nc.tensor.matmul(
                            wop[:], chlT[:, hsl, :, 1], woh_sb[:, j, :, esl],
                            start=False, stop=False, perf_mode=PM.DoubleRow)
                        nc.tensor.matmul(
                            wop[:], chlT[:, hsl, :, 0], wol_sb[:, j, :, esl],
                            start=False, stop=(j == NPAIR - 1),
                            perf_mode=PM.DoubleRow)
                    out_sb = work.tile([128, 512], BF16, tag="osb", bufs=3)
                    nc.scalar.activation(out_sb[:], wop[:], AF.Copy,
                                         scale=OUT_DESCALE)
                    nc.sync.dma_start(
                        out=out[sc * 128:(sc + 1) * 128, esl], in_=out_sb[:])
                    drain_q(1)

            if DEBUG:
                nc.sync.dma_start(out=dbg_pqT[:], in_=pqT[:])
    nc.compile()
    return nc


def get_module():
    if "nc" not in _CACHED:
        _CACHED["nc"] = build_module()
    return _CACHED["nc"]


def _e4(a, scale):
    return (np.ascontiguousarray(a, dtype=np.float32) * scale).astype(E4NP)


def _bf(a):
    return np.ascontiguousarray(a, dtype=np.float32).astype(BFNP)


def prepare_in_maps(inputs, Wq, Wk, Wv, Wo, bo):
    """Host-side shard + layout prep. Returns per-core input maps."""
    Wq = np.asarray(Wq, dtype=np.float32)
    Wk = np.asarray(Wk, dtype=np.float32)
    Wv = np.asarray(Wv, dtype=np.float32)
    Wo = np.asarray(Wo, dtype=np.float32)
    # w8: [64, H, 2, 256] = (Wq | Wk) * 256
    wq_p = np.transpose(Wq.reshape(H, 2, 64, D), (2, 0, 1, 3))
    wk_p = np.transpose(Wk.reshape(H, 2, 64, D), (2, 0, 1, 3))
    w8_p = _e4(np.concatenate([wq_p, wk_p], axis=3), SWQ)
    # wv: [128, H*128]
    wv_p = _bf(np.transpose(Wv, (1, 0, 2)).reshape(D, H * D))
    # wo hi/lo: [128, NPAIR, 2, E] with residual split at scale SWO
    wo_r = np.transpose(Wo.reshape(NPAIR, 2, D, E), (2, 0, 1, 3))
    wo_hi = _e4(wo_r, SWO)
    wo_lo = (wo_r * SWO - wo_hi.astype(np.float32)).astype(E4NP)
    in_maps = []
    for b in range(N_CORES):
        xb = np.asarray(inputs[b], dtype=np.float32)
        # x8: [64, H, 2, S]: x8[p,h,i,s] = 16*x[s, h*128+i*64+p]
        x8_p = _e4(np.transpose(xb.reshape(S, H, 2, 64), (3, 1, 2, 0)), SX)
        # xn packed per head: xn[h][p, c*128+j] = x[c*128+p, h*128+j]
        xn_p = _bf(np.transpose(xb.reshape(NCH, 128, H, D),
                                (2, 1, 0, 3)).reshape(H, 128, NCH * D))
        in_maps.append({"x8": x8_p, "xn": xn_p, "w8": w8_p, "wv": wv_p,
                        "wo_hi": wo_hi, "wo_lo": wo_lo})
    return in_maps


def kernel(inputs, Wq, Wk, Wv, Wo, bo):
    B = inputs.shape[0]
    assert B == N_CORES and inputs.shape[1:] == (S, E)
    nc = get_module()
    in_maps = prepare_in_maps(inputs, Wq, Wk, Wv, Wo, bo)
    res = run_bass_kernel_spmd(nc, in_maps, list(range(N_CORES)))
    outs = np.stack([res.results[b]["out"].astype(np.float32)
                     for b in range(N_CORES)], axis=0)
    return (outs + np.asarray(bo, dtype=np.float32)[None, None, :]).astype(
        np.float32)


# revision 12
# speedup vs baseline: 1.2231x; 1.1645x over previous
"""Trainium2 Bass kernel for nn_MultiHeadLinearAttention.

Full-input contract: kernel(**inputs) takes the unsharded numpy inputs and
returns the full output. Internally: data-parallel over batch across the 8
NeuronCores (B == 8, one batch element per core), no collectives.

Per-core math (S=2048, E=2048, H=16, d=128):
  q/k projections: fp8(e4m3) DoubleRow matmuls, contraction d=128 packed as
    two 64-partition K-tiles (x and W pre-packed [64, 2, *] on host, scales
    x*16, W*256 so the psum holds q*2^12). A tiny fp8 "ones" matmul preloads
    the psum with 2^12 so it holds (q+1)*2^12, which lets
    phi(x) = elu(x)+1 = min(exp(x), max(x+1, 1)) be computed with one Act op
    (Exp with scale 2^-12, bias -1) + one DVE tensor_scalar
    ((in max 4096) * 2^-12) + one DVE min.
  Gram trick (bf16): G[d',d] = sum_s xh[s,d'] pk[s,d]; kv = G^T @ Wv; ksum
    fused as psum col 128.  num/den: bf16; den for all 16 heads of an
    s-chunk lands in one [128,16] psum tile -> single reciprocal.
  ctx: stored as fp8 hi+lo residual pair (scale 2^10), byte-interleaved
    [s, h, d, 2] so ONE uint16-view blocked DMA transpose yields
    [d, h, s, 2] with hi/lo selectable by byte offset.
  Wo stage: fp8 DoubleRow, 3 instructions per head-pair per e-block:
    (c_hi+c_lo) @ w_hi (2 instrs) + c_hi @ w_lo (1 instr) -- 0.75 PE
    cycles per K=128 column vs 1.0 for bf16, with ~bf16-level accuracy
    (Wo pre-split into hi+lo e4m3 on host, scale 2^10).
  Output: psum * 2^-20 -> bf16 SBUF -> DRAM; host upcasts and adds bias.
"""

import numpy as np
import ml_dtypes

import concourse.bass as bass
import concourse.mybir as mybir
import concourse.tile as tile
from concourse import bacc
from concourse.bass_utils import run_bass_kernel_spmd

S = 2048
E = 2048
H = 16
D = 128
N_CORES = 8
NCH = S // 128  # 16 s-chunks
NPAIR = H // 2  # 8 head pairs

F32 = mybir.dt.float32
BF16 = mybir.dt.bfloat16
FP8 = mybir.dt.float8e4
U16 = mybir.dt.uint16
AF = mybir.ActivationFunctionType
ALU = mybir.AluOpType
PM = mybir.MatmulPerfMode

E4NP = ml_dtypes.float8_e4m3
BFNP = ml_dtypes.bfloat16

SX = 16.0       # x -> fp8 scale
SWQ = 256.0     # Wq/Wk -> fp8 scale (psum = q * 2^12)
SC = 1024.0     # ctx -> fp8 scale
SWO = 1024.0    # Wo -> fp8 scale
OUT_DESCALE = 1.0 / (SC * SWO)

_CACHED = {}
DEBUG = False


def build_module():
    nc = bacc.Bacc("TRN2", target_bir_lowering=False, debug=False,
                   num_devices=N_CORES)

    x8 = nc.dram_tensor("x8", [64, H, 2, S], FP8, kind="ExternalInput")
    xn = nc.dram_tensor("xn", [H, 128, NCH * 128], BF16,
                        kind="ExternalInput")
    w8 = nc.dram_tensor("w8", [64, H, 2, 256], FP8, kind="ExternalInput")
    wv = nc.dram_tensor("wv", [128, H * 128], BF16, kind="ExternalInput")
    wo_hi = nc.dram_tensor("wo_hi", [128, NPAIR, 2, E], FP8,
                           kind="ExternalInput")
    wo_lo = nc.dram_tensor("wo_lo", [128, NPAIR, 2, E], FP8,
                           kind="ExternalInput")
    out = nc.dram_tensor("out", [S, E], BF16, kind="ExternalOutput")
    if DEBUG:
        dbg_pqT = nc.dram_tensor("dbg_pqT", [128, H * S], BF16,
                                 kind="ExternalOutput")
        dbg_kv = nc.dram_tensor("dbg_kv", [128, H, 128], BF16,
                                kind="ExternalOutput")
        dbg_ksd = nc.dram_tensor("dbg_ksd", [128, H], BF16,
                                 kind="ExternalOutput")
        dbg_inv = nc.dram_tensor("dbg_inv", [128, 16], F32,
                                 kind="ExternalOutput")
        dbg_chl = nc.dram_tensor("dbg_chl", [128, H, 128, 2], FP8,
                                 kind="ExternalOutput")
        dbg_chlT = nc.dram_tensor("dbg_chlT", [128, H, 128, 2], FP8,
                                  kind="ExternalOutput")
        dbg_g = nc.dram_tensor("dbg_g", [128, 128], BF16,
                               kind="ExternalOutput")

    with tile.TileContext(nc) as tc:
        with (
            tc.tile_pool(name="const", bufs=1) as const,
            tc.tile_pool(name="work", bufs=2) as work,
            tc.tile_pool(name="psum", bufs=2, space="PSUM") as psum,
        ):
            # ---------------- constants / persistent tiles ----------------
            wv_sb = const.tile([128, H * 128], BF16)
            nc.sync.dma_start(out=wv_sb[:], in_=wv[:])
            woh_sb = const.tile([128, NPAIR, 2, E], FP8)
            nc.sync.dma_start(out=woh_sb[:], in_=wo_hi[:])
            wol_sb = const.tile([128, NPAIR, 2, E], FP8)
            nc.sync.dma_start(out=wol_sb[:], in_=wo_lo[:])
            w8_sb = const.tile([64, H, 2, 256], FP8)
            nc.sync.dma_start(out=w8_sb[:], in_=w8[:])

            pre_l = const.tile([1, 2, 128], FP8)
            nc.vector.memset(pre_l[:], 32.0)
            pre_r = const.tile([1, 2, 512], FP8)
            nc.vector.memset(pre_r[:], 64.0)
            neg1 = const.tile([128, 1], F32)
            nc.vector.memset(neg1[:], -1.0)
            ones = const.tile([128, 1], BF16)
            nc.vector.memset(ones[:], 1.0)

            pqT = const.tile([128, H * S], BF16)       # all heads, transposed
            kv_all = const.tile([128, H, 128], BF16)   # kv per head
            ksd = const.tile([128, H], BF16)           # ksum * 2^-10 per head

            # ---------------- helpers ----------------
            def phi(src_psum, dst, n):
                """psum holds (v+1)*2^12; writes phi(v) [128, n] bf16."""
                e = work.tile([128, 512], BF16, tag="e", bufs=2)
                nc.scalar.activation(e[:, 0:n], src_psum[:, 0:n], AF.Exp,
                                     bias=neg1[:, 0:1], scale=2.0 ** -12)
                t = work.tile([128, 512], BF16, tag="t", bufs=2)
                nc.vector.tensor_scalar(t[:, 0:n], src_psum[:, 0:n], 4096.0,
                                        2.0 ** -12, ALU.max, ALU.mult)
                nc.vector.tensor_tensor(dst, e[:, 0:n], t[:, 0:n], ALU.min)

            def preload(p, n):
                nc.tensor.matmul(p[:, 0:n], pre_l[:], pre_r[:, :, 0:n],
                                 start=True, stop=False,
                                 perf_mode=PM.DoubleRow)

            # deferred q quarters: (h, quarter) for quarters 1..3
            q_queue = [(h, qt) for qt in (1, 2, 3) for h in range(H)]
            q_queue.reverse()
            q_pf = list(q_queue)
            q_pending = []

            def prefetch_q_quarter():
                h, qt = q_pf.pop()
                xq = work.tile([64, 2, 512], FP8, tag="xq", bufs=3)
                nc.sync.dma_start(
                    out=xq[:], in_=x8[:, h, :, qt * 512:(qt + 1) * 512])
                q_pending.append(xq)

            def emit_q_quarter(h, qt):
                xq = q_pending.pop(0)
                qp = psum.tile([128, 512], F32, tag="qk", bufs=2)
                preload(qp, 512)
                nc.tensor.matmul(
                    qp[:], w8_sb[:, h, :, 0:128], xq[:],
                    start=False, stop=True, perf_mode=PM.DoubleRow)
                phi(qp, pqT[:, h * S + qt * 512:h * S + (qt + 1) * 512], 512)

            def drain_q(k):
                for _ in range(k):
                    while q_pf and len(q_pending) < 2:
                        prefetch_q_quarter()
                    if q_queue:
                        emit_q_quarter(*q_queue.pop())

            # ---------------- pass A: per head ----------------
            for h in range(H):
                x8_h = work.tile([64, 2, S], FP8, tag="x8h", bufs=2)
                nc.sync.dma_start(out=x8_h[:], in_=x8[:, h])
                xn_h = work.tile([128, NCH, 128], BF16, tag="xn", bufs=2)
                nc.sync.dma_start(
                    out=xn_h[:], in_=xn[h].rearrange("p (c j) -> p c j",
                                                     j=128))

                # k projection (natural layout) + phi -> pk
                pk = work.tile([128, NCH, 128], BF16, tag="pk", bufs=2)
                for g in range(4):
                    kp = psum.tile([128, 512], F32, tag="qk", bufs=2)
                    preload(kp, 512)
                    for c in range(4):
                        ch = g * 4 + c
                        nc.tensor.matmul(
                            kp[:, c * 128:(c + 1) * 128],
                            x8_h[:, :, ch * 128:(ch + 1) * 128],
                            w8_sb[:, h, :, 128:256],
                            start=False, stop=(c == 3),
                            perf_mode=PM.DoubleRow)
                    phi(kp, pk[:, 4 * g:4 * g + 4, :].rearrange(
                        "p c j -> p (c j)"), 512)

                # q projection quarter 0 (transposed layout)
                qp = psum.tile([128, 512], F32, tag="qk", bufs=2)
                preload(qp, 512)
                nc.tensor.matmul(qp[:], w8_sb[:, h, :, 0:128],
                                 x8_h[:, :, 0:512], start=False, stop=True,
                                 perf_mode=PM.DoubleRow)
                phi(qp, pqT[:, h * S:h * S + 512], 512)

                # Gram G + ksum (bf16)
                gp_t = psum.tile([128, 128], F32, tag="g", bufs=1)
                gp = gp_t[:]
                kvp_t = psum.tile([128, 129], F32, tag="kv", bufs=1)
                kvp = kvp_t[:]
                for c in range(NCH):
                    nc.tensor.matmul(gp, xn_h[:, c, :], pk[:, c, :],
                                     start=(c == 0), stop=(c == NCH - 1),
                                     skip_group_check=True)
                    nc.tensor.matmul(kvp[:, 128:129], pk[:, c, :], ones[:],
                                     start=(c == 0), stop=(c == NCH - 1),
                                     skip_group_check=True)
                g_sb = work.tile([128, 128], BF16, tag="gsb", bufs=2)
                nc.scalar.activation(g_sb[:], gp, AF.Copy)
                if DEBUG and h == 0:
                    nc.sync.dma_start(out=dbg_g[:], in_=g_sb[:])
                nc.tensor.matmul(kvp[:, 0:128], g_sb[:],
                                 wv_sb[:, h * 128:(h + 1) * 128],
                                 start=True, stop=True, skip_group_check=True)
                nc.scalar.activation(kv_all[:, h, :], kvp[:, 0:128], AF.Copy)
                nc.scalar.activation(ksd[:, h:h + 1], kvp[:, 128:129],
                                     AF.Copy, scale=2.0 ** -10)

            if DEBUG:
                nc.sync.dma_start(out=dbg_pqT[:, 0:512], in_=pqT[:, 0:512])
                nc.sync.dma_start(out=dbg_kv[:], in_=kv_all[:])
                nc.sync.dma_start(out=dbg_ksd[:], in_=ksd[:])
            # ---------------- pass B: per s-chunk ----------------
            chlT_tiles = {}

            def emit_wo(wsc):
                chlT_w = chlT_tiles.pop(wsc)
                for eb in range(4):
                    wop = psum.tile([128, 512], F32, tag="wo", bufs=2)
                    esl = slice(eb * 512, (eb + 1) * 512)
                    for j in range(NPAIR):
                        hsl = slice(2 * j, 2 * j + 2)
                        nc.tensor.matmul(
                            wop[:], chlT_w[:, hsl, :, 0],
                            woh_sb[:, j, :, esl], start=(j == 0), stop=False,
                            perf_mode=PM.DoubleRow)
                        nc.tensor.matmul(
                            wop[:], chlT_w[:, hsl, :, 1],
                            woh_sb[:, j, :, esl], start=False, stop=False,
                            perf_mode=PM.DoubleRow)
                        nc.tensor.matmul(
                            wop[:], chlT_w[:, hsl, :, 0],
                            wol_sb[:, j, :, esl], start=False,
                            stop=(j == NPAIR - 1), perf_mode=PM.DoubleRow)
                    out_sb = work.tile([128, 512], BF16, tag="osb", bufs=3)
                    nc.scalar.activation(out_sb[:], wop[:], AF.Copy,
                                         scale=OUT_DESCALE)
                    nc.sync.dma_start(
                        out=out[wsc * 128:(wsc + 1) * 128, esl], in_=out_sb[:])
                    drain_q(1)

            for sc in range(NCH):
                # den for all heads -> one [128, 16] psum; then reciprocal
                den_t = psum.tile([128, 129], F32, tag="kv", bufs=1)
                denp = den_t[:, 0:16]
                for h in range(H):
                    nc.tensor.matmul(
                        denp[:, h:h + 1],
                        pqT[:, h * S + sc * 128:h * S + (sc + 1) * 128],
                        ksd[:, h:h + 1], start=True, stop=True,
                        skip_group_check=True)
                invsc = work.tile([128, 16], F32, tag="inv", bufs=2)
                nc.vector.reciprocal(invsc[:], denp)
                if DEBUG and sc == 0:
                    nc.sync.dma_start(out=dbg_inv[:], in_=invsc[:])  # = 1024/den

                drain_q(1)

                # num + ctx scale to bf16 per head (DVE/Act alternating),
                # then one batched fp8 hi-cast + one batched lo-subtract
                chl = work.tile([128, H, 128, 2], FP8, tag="chl", bufs=2)
                cbf = work.tile([128, H, 128], BF16, tag="cbf", bufs=2)
                for h in range(H):
                    ndp = psum.tile([128, 128], F32, tag="nd", bufs=2)
                    nc.tensor.matmul(
                        ndp[:],
                        pqT[:, h * S + sc * 128:h * S + (sc + 1) * 128],
                        kv_all[:, h, :], start=True, stop=True)
                    if h % 2 == 0:
                        nc.vector.tensor_scalar(cbf[:, h, :], ndp[:],
                                                invsc[:, h:h + 1], None,
                                                ALU.mult)
                    else:
                        nc.scalar.activation(cbf[:, h, :], ndp[:], AF.Copy,
                                             scale=invsc[:, h:h + 1])
                    if h in (5, 11):
                        drain_q(1)
                nc.scalar.activation(chl[:, :, :, 0], cbf[:], AF.Copy)
                nc.vector.tensor_tensor(chl[:, :, :, 1], cbf[:],
                                        chl[:, :, :, 0], ALU.subtract)

                # one blocked u16 transpose: [s,(h d)] -> [d, h, s] pairs
                if DEBUG and sc == 0:
                    nc.sync.dma_start(out=dbg_chl[:], in_=chl[:])
                chlT = work.tile([128, H, 128, 2], FP8, tag="chlT", bufs=2)
                nc.scalar.dma_start(
                    out=chlT[:].rearrange("p h s two -> p h (s two)").bitcast(
                        U16),
                    in_=chl[:].rearrange("p h d two -> p (h d two)").bitcast(
                        U16),
                    transpose=True)

                if DEBUG and sc == 0:
                    nc.sync.dma_start(out=dbg_chlT[:], in_=chlT[:])
                chlT_tiles[sc] = chlT
                if sc > 0:
                    emit_wo(sc - 1)
                if sc == NCH - 1:
                    emit_wo(sc)

            if DEBUG:
                nc.sync.dma_start(out=dbg_pqT[:], in_=pqT[:])
    nc.compile()
    return nc


def get_module():
    if "nc" not in _CACHED:
        _CACHED["nc"] = build_module()
    return _CACHED["nc"]


def _e4(a, scale):
    return (np.ascontiguousarray(a, dtype=np.float32) * scale).astype(E4NP)


def _bf(a):
    return np.ascontiguousarray(a, dtype=np.float32).astype(BFNP)


def prepare_in_maps(inputs, Wq, Wk, Wv, Wo, bo):
    """Host-side shard + layout prep. Returns per-core input maps."""
    Wq = np.asarray(Wq, dtype=np.float32)
    Wk = np.asarray(Wk, dtype=np.float32)
    Wv = np.asarray(Wv, dtype=np.float32)
    Wo = np.asarray(Wo, dtype=np.float32)
    # w8: [64, H, 2, 256] = (Wq | Wk) * 256
    wq_p = np.transpose(Wq.reshape(H, 2, 64, D), (2, 0, 1, 3))
    wk_p = np.transpose(Wk.reshape(H, 2, 64, D), (2, 0, 1, 3))
    w8_p = _e4(np.concatenate([wq_p, wk_p], axis=3), SWQ)
    # wv: [128, H*128]
    wv_p = _bf(np.transpose(Wv, (1, 0, 2)).reshape(D, H * D))
    # wo hi/lo: [128, NPAIR, 2, E] with residual split at scale SWO
    wo_r = np.transpose(Wo.reshape(NPAIR, 2, D, E), (2, 0, 1, 3))
    wo_hi = _e4(wo_r, SWO)
    wo_lo = (wo_r * SWO - wo_hi.astype(np.float32)).astype(E4NP)
    in_maps = []
    for b in range(N_CORES):
        xb = np.asarray(inputs[b], dtype=np.float32)
        # x8: [64, H, 2, S]: x8[p,h,i,s] = 16*x[s, h*128+i*64+p]
        x8_p = _e4(np.transpose(xb.reshape(S, H, 2, 64), (3, 1, 2, 0)), SX)
        # xn packed per head: xn[h][p, c*128+j] = x[c*128+p, h*128+j]
        xn_p = _bf(np.transpose(xb.reshape(NCH, 128, H, D),
                                (2, 1, 0, 3)).reshape(H, 128, NCH * D))
        in_maps.append({"x8": x8_p, "xn": xn_p, "w8": w8_p, "wv": wv_p,
                        "wo_hi": wo_hi, "wo_lo": wo_lo})
    return in_maps


def kernel(inputs, Wq, Wk, Wv, Wo, bo):
    B = inputs.shape[0]
    assert B == N_CORES and inputs.shape[1:] == (S, E)
    nc = get_module()
    in_maps = prepare_in_maps(inputs, Wq, Wk, Wv, Wo, bo)
    res = run_bass_kernel_spmd(nc, in_maps, list(range(N_CORES)))
    outs = np.stack([res.results[b]["out"].astype(np.float32)
                     for b in range(N_CORES)], axis=0)
    return (outs + np.asarray(bo, dtype=np.float32)[None, None, :]).astype(
        np.float32)


# revision 13
# speedup vs baseline: 1.2373x; 1.0116x over previous
"""Trainium2 Bass kernel for nn_MultiHeadLinearAttention.

Full-input contract: kernel(**inputs) takes the unsharded numpy inputs and
returns the full output. Internally: data-parallel over batch across the 8
NeuronCores (B == 8, one batch element per core), no collectives.

Per-core math (S=2048, E=2048, H=16, d=128):
  q/k projections: fp8(e4m3) DoubleRow matmuls, contraction d=128 packed as
    two 64-partition K-tiles (x and W pre-packed [64, 2, *] on host, scales
    x*16, W*256 so the psum holds q*2^12). A tiny fp8 "ones" matmul preloads
    the psum with 2^12 so it holds (q+1)*2^12, which lets
    phi(x) = elu(x)+1 = min(exp(x), max(x+1, 1)) be computed with one Act op
    (Exp with scale 2^-12, bias -1) + one DVE tensor_scalar
    ((in max 4096) * 2^-12) + one DVE min.
  Gram trick (bf16): G[d',d] = sum_s xh[s,d'] pk[s,d]; kv = G^T @ Wv; ksum
    fused as psum col 128.  num/den: bf16; den for all 16 heads of an
    s-chunk lands in one [128,16] psum tile -> single reciprocal.
  ctx: stored as fp8 hi+lo residual pair (scale 2^10), byte-interleaved
    [s, h, d, 2] so ONE uint16-view blocked DMA transpose yields
    [d, h, s, 2] with hi/lo selectable by byte offset.
  Wo stage: fp8 DoubleRow, 3 instructions per head-pair per e-block:
    (c_hi+c_lo) @ w_hi (2 instrs) + c_hi @ w_lo (1 instr) -- 0.75 PE
    cycles per K=128 column vs 1.0 for bf16, with ~bf16-level accuracy
    (Wo pre-split into hi+lo e4m3 on host, scale 2^10).
  Output: psum * 2^-20 -> bf16 SBUF -> DRAM; host upcasts and adds bias.
"""

import numpy as np
import ml_dtypes

import concourse.bass as bass
import concourse.mybir as mybir
import concourse.tile as tile
from concourse import bacc
from concourse.bass_utils import run_bass_kernel_spmd

S = 2048
E = 2048
H = 16
D = 128
N_CORES = 8
NCH = S // 128  # 16 s-chunks
NPAIR = H // 2  # 8 head pairs

F32 = mybir.dt.float32
BF16 = mybir.dt.bfloat16
FP8 = mybir.dt.float8e4
U16 = mybir.dt.uint16
AF = mybir.ActivationFunctionType
ALU = mybir.AluOpType
PM = mybir.MatmulPerfMode

E4NP = ml_dtypes.float8_e4m3
BFNP = ml_dtypes.bfloat16

SX = 16.0       # x -> fp8 scale
SWQ = 256.0     # Wq/Wk -> fp8 scale (psum = q * 2^12)
SC = 1024.0     # ctx -> fp8 scale
SWO = 1024.0    # Wo -> fp8 scale
OUT_DESCALE = 1.0 / (SC * SWO)

_CACHED = {}
DEBUG = False


def build_module():
    nc = bacc.Bacc("TRN2", target_bir_lowering=False, debug=False,
                   num_devices=N_CORES)

    x8 = nc.dram_tensor("x8", [64, H, 2, S], FP8, kind="ExternalInput")
    xn = nc.dram_tensor("xn", [H, 128, NCH * 128], BF16,
                        kind="ExternalInput")
    w8 = nc.dram_tensor("w8", [64, H, 2, 256], FP8, kind="ExternalInput")
    wv = nc.dram_tensor("wv", [128, H * 128], BF16, kind="ExternalInput")
    wo_hi = nc.dram_tensor("wo_hi", [128, NPAIR, 2, E], FP8,
                           kind="ExternalInput")
    wo_lo = nc.dram_tensor("wo_lo", [128, NPAIR, 2, E], FP8,
                           kind="ExternalInput")
    out = nc.dram_tensor("out", [S, E], BF16, kind="ExternalOutput")
    if DEBUG:
        dbg_pqT = nc.dram_tensor("dbg_pqT", [128, H * S], BF16,
                                 kind="ExternalOutput")
        dbg_kv = nc.dram_tensor("dbg_kv", [128, H, 128], BF16,
                                kind="ExternalOutput")
        dbg_ksd = nc.dram_tensor("dbg_ksd", [128, H], BF16,
                                 kind="ExternalOutput")
        dbg_inv = nc.dram_tensor("dbg_inv", [128, 16], F32,
                                 kind="ExternalOutput")
        dbg_chl = nc.dram_tensor("dbg_chl", [128, H, 128, 2], FP8,
                                 kind="ExternalOutput")
        dbg_chlT = nc.dram_tensor("dbg_chlT", [128, H, 128, 2], FP8,
                                  kind="ExternalOutput")
        dbg_g = nc.dram_tensor("dbg_g", [128, 128], BF16,
                               kind="ExternalOutput")

    with tile.TileContext(nc) as tc:
        with (
            tc.tile_pool(name="const", bufs=1) as const,
            tc.tile_pool(name="work", bufs=2) as work,
            tc.tile_pool(name="psum", bufs=2, space="PSUM") as psum,
        ):
            # ---------------- constants / persistent tiles ----------------
            w8_sb = const.tile([64, H, 2, 256], FP8)
            nc.sync.dma_start(out=w8_sb[:], in_=w8[:])
            wv_sb = const.tile([128, H * 128], BF16)
            woh_sb = const.tile([128, NPAIR, 2, E], FP8)
            wol_sb = const.tile([128, NPAIR, 2, E], FP8)

            pre_l = const.tile([1, 2, 128], FP8)
            nc.vector.memset(pre_l[:], 32.0)
            pre_r = const.tile([1, 2, 512], FP8)
            nc.vector.memset(pre_r[:], 64.0)
            neg1 = const.tile([128, 1], F32)
            nc.vector.memset(neg1[:], -1.0)
            ones = const.tile([128, 1], BF16)
            nc.vector.memset(ones[:], 1.0)

            pqT = const.tile([128, H * S], BF16)       # all heads, transposed
            kv_all = const.tile([128, H, 128], BF16)   # kv per head
            ksd = const.tile([128, H], BF16)           # ksum * 2^-10 per head

            # ---------------- helpers ----------------
            def phi(src_psum, dst, n):
                """psum holds (v+1)*2^12; writes phi(v) [128, n] bf16."""
                e = work.tile([128, 512], BF16, tag="e", bufs=2)
                nc.scalar.activation(e[:, 0:n], src_psum[:, 0:n], AF.Exp,
                                     bias=neg1[:, 0:1], scale=2.0 ** -12)
                t = work.tile([128, 512], BF16, tag="t", bufs=2)
                nc.vector.tensor_scalar(t[:, 0:n], src_psum[:, 0:n], 4096.0,
                                        2.0 ** -12, ALU.max, ALU.mult)
                nc.vector.tensor_tensor(dst, e[:, 0:n], t[:, 0:n], ALU.min)

            def preload(p, n):
                nc.tensor.matmul(p[:, 0:n], pre_l[:], pre_r[:, :, 0:n],
                                 start=True, stop=False,
                                 perf_mode=PM.DoubleRow)

            # deferred q quarters: (h, quarter) for quarters 1..3
            q_queue = [(h, qt) for qt in (1, 2, 3) for h in range(H)]
            q_queue.reverse()
            q_pf = list(q_queue)
            q_pending = []

            def prefetch_q_quarter():
                h, qt = q_pf.pop()
                xq = work.tile([64, 2, 512], FP8, tag="xq", bufs=5)
                nc.sync.dma_start(
                    out=xq[:], in_=x8[:, h, :, qt * 512:(qt + 1) * 512])
                q_pending.append(xq)

            def emit_q_quarter(h, qt):
                xq = q_pending.pop(0)
                qp = psum.tile([128, 512], F32, tag="qk", bufs=2)
                preload(qp, 512)
                nc.tensor.matmul(
                    qp[:], w8_sb[:, h, :, 0:128], xq[:],
                    start=False, stop=True, perf_mode=PM.DoubleRow)
                phi(qp, pqT[:, h * S + qt * 512:h * S + (qt + 1) * 512], 512)

            def drain_q(k):
                for _ in range(k):
                    while q_pf and len(q_pending) < 4:
                        prefetch_q_quarter()
                    if q_queue:
                        emit_q_quarter(*q_queue.pop())

            # ---------------- pass A: per head ----------------
            for h in range(H):
                x8_h = work.tile([64, 2, S], FP8, tag="x8h", bufs=2)
                nc.sync.dma_start(out=x8_h[:], in_=x8[:, h])
                if h == 0:
                    nc.sync.dma_start(out=wv_sb[:], in_=wv[:])
                if h == 1:
                    nc.sync.dma_start(out=woh_sb[:], in_=wo_hi[:])
                if h == 2:
                    nc.sync.dma_start(out=wol_sb[:], in_=wo_lo[:])
                xn_h = work.tile([128, NCH, 128], BF16, tag="xn", bufs=2)
                nc.sync.dma_start(
                    out=xn_h[:], in_=xn[h].rearrange("p (c j) -> p c j",
                                                     j=128))

                # k projection (natural layout) + phi -> pk
                pk = work.tile([128, NCH, 128], BF16, tag="pk", bufs=2)
                for g in range(4):
                    kp = psum.tile([128, 512], F32, tag="qk", bufs=2)
                    preload(kp, 512)
                    for c in range(4):
                        ch = g * 4 + c
                        nc.tensor.matmul(
                            kp[:, c * 128:(c + 1) * 128],
                            x8_h[:, :, ch * 128:(ch + 1) * 128],
                            w8_sb[:, h, :, 128:256],
                            start=False, stop=(c == 3),
                            perf_mode=PM.DoubleRow)
                    phi(kp, pk[:, 4 * g:4 * g + 4, :].rearrange(
                        "p c j -> p (c j)"), 512)

                # q projection quarter 0 (transposed layout)
                qp = psum.tile([128, 512], F32, tag="qk", bufs=2)
                preload(qp, 512)
                nc.tensor.matmul(qp[:], w8_sb[:, h, :, 0:128],
                                 x8_h[:, :, 0:512], start=False, stop=True,
                                 perf_mode=PM.DoubleRow)
                phi(qp, pqT[:, h * S:h * S + 512], 512)

                # Gram G + ksum (bf16)
                gp_t = psum.tile([128, 128], F32, tag="g", bufs=1)
                gp = gp_t[:]
                kvp_t = psum.tile([128, 129], F32, tag="kv", bufs=1)
                kvp = kvp_t[:]
                for c in range(NCH):
                    nc.tensor.matmul(gp, xn_h[:, c, :], pk[:, c, :],
                                     start=(c == 0), stop=(c == NCH - 1),
                                     skip_group_check=True)
                    nc.tensor.matmul(kvp[:, 128:129], pk[:, c, :], ones[:],
                                     start=(c == 0), stop=(c == NCH - 1),
                                     skip_group_check=True)
                g_sb = work.tile([128, 128], BF16, tag="gsb", bufs=2)
                nc.scalar.activation(g_sb[:], gp, AF.Copy)
                if DEBUG and h == 0:
                    nc.sync.dma_start(out=dbg_g[:], in_=g_sb[:])
                nc.tensor.matmul(kvp[:, 0:128], g_sb[:],
                                 wv_sb[:, h * 128:(h + 1) * 128],
                                 start=True, stop=True, skip_group_check=True)
                nc.scalar.activation(kv_all[:, h, :], kvp[:, 0:128], AF.Copy)
                nc.scalar.activation(ksd[:, h:h + 1], kvp[:, 128:129],
                                     AF.Copy, scale=2.0 ** -10)

            if DEBUG:
                nc.sync.dma_start(out=dbg_pqT[:, 0:512], in_=pqT[:, 0:512])
                nc.sync.dma_start(out=dbg_kv[:], in_=kv_all[:])
                nc.sync.dma_start(out=dbg_ksd[:], in_=ksd[:])
            # ---------------- pass B: per s-chunk ----------------
            chlT_tiles = {}

            def emit_wo(wsc):
                chlT_w = chlT_tiles.pop(wsc)
                for eb in range(4):
                    wop = psum.tile([128, 512], F32, tag="wo", bufs=2)
                    esl = slice(eb * 512, (eb + 1) * 512)
                    for j in range(NPAIR):
                        hsl = slice(2 * j, 2 * j + 2)
                        nc.tensor.matmul(
                            wop[:], chlT_w[:, hsl, :, 0],
                            woh_sb[:, j, :, esl], start=(j == 0), stop=False,
                            perf_mode=PM.DoubleRow)
                        nc.tensor.matmul(
                            wop[:], chlT_w[:, hsl, :, 1],
                            woh_sb[:, j, :, esl], start=False, stop=False,
                            perf_mode=PM.DoubleRow)
                        nc.tensor.matmul(
                            wop[:], chlT_w[:, hsl, :, 0],
                            wol_sb[:, j, :, esl], start=False,
                            stop=(j == NPAIR - 1), perf_mode=PM.DoubleRow)
                    out_sb = work.tile([128, 512], BF16, tag="osb", bufs=3)
                    nc.scalar.activation(out_sb[:], wop[:], AF.Copy,
                                         scale=OUT_DESCALE)
                    nc.sync.dma_start(
                        out=out[wsc * 128:(wsc + 1) * 128, esl], in_=out_sb[:])
                    drain_q(1)

            for sc in range(NCH):
                # den for all heads -> one [128, 16] psum; then reciprocal
                den_t = psum.tile([128, 129], F32, tag="kv", bufs=1)
                denp = den_t[:, 0:16]
                for h in range(H):
                    nc.tensor.matmul(
                        denp[:, h:h + 1],
                        pqT[:, h * S + sc * 128:h * S + (sc + 1) * 128],
                        ksd[:, h:h + 1], start=True, stop=True,
                        skip_group_check=True)
                invsc = work.tile([128, 16], F32, tag="inv", bufs=2)
                nc.vector.reciprocal(invsc[:], denp)
                if DEBUG and sc == 0:
                    nc.sync.dma_start(out=dbg_inv[:], in_=invsc[:])  # = 1024/den

                drain_q(1)

                # num + ctx scale to bf16 per head (DVE/Act alternating),
                # then one batched fp8 hi-cast + one batched lo-subtract
                chl = work.tile([128, H, 128, 2], FP8, tag="chl", bufs=2)
                cbf = work.tile([128, H, 128], BF16, tag="cbf", bufs=2)
                for h in range(H):
                    ndp = psum.tile([128, 128], F32, tag="nd", bufs=2)
                    nc.tensor.matmul(
                        ndp[:],
                        pqT[:, h * S + sc * 128:h * S + (sc + 1) * 128],
                        kv_all[:, h, :], start=True, stop=True)
                    if h % 2 == 0:
                        nc.vector.tensor_scalar(cbf[:, h, :], ndp[:],
                                                invsc[:, h:h + 1], None,
                                                ALU.mult)
                    else:
                        nc.scalar.activation(cbf[:, h, :], ndp[:], AF.Copy,
                                             scale=invsc[:, h:h + 1])
                    if h in (5, 11):
                        drain_q(1)
                nc.scalar.activation(chl[:, :, :, 0], cbf[:], AF.Copy)
                nc.vector.tensor_tensor(chl[:, :, :, 1], cbf[:],
                                        chl[:, :, :, 0], ALU.subtract)

                # one blocked u16 transpose: [s,(h d)] -> [d, h, s] pairs
                if DEBUG and sc == 0:
                    nc.sync.dma_start(out=dbg_chl[:], in_=chl[:])
                chlT = work.tile([128, H, 128, 2], FP8, tag="chlT", bufs=2)
                nc.scalar.dma_start(
                    out=chlT[:].rearrange("p h s two -> p h (s two)").bitcast(
                        U16),
                    in_=chl[:].rearrange("p h d two -> p (h d two)").bitcast(
                        U16),
                    transpose=True)

                if DEBUG and sc == 0:
                    nc.sync.dma_start(out=dbg_chlT[:], in_=chlT[:])
                chlT_tiles[sc] = chlT
                if sc > 0:
                    emit_wo(sc - 1)
                if sc == NCH - 1:
                    emit_wo(sc)

            if DEBUG:
                nc.sync.dma_start(out=dbg_pqT[:], in_=pqT[:])
    nc.compile()
    return nc


def get_module():
    if "nc" not in _CACHED:
        _CACHED["nc"] = build_module()
    return _CACHED["nc"]


def _e4(a, scale):
    return (np.ascontiguousarray(a, dtype=np.float32) * scale).astype(E4NP)


def _bf(a):
    return np.ascontiguousarray(a, dtype=np.float32).astype(BFNP)


def prepare_in_maps(inputs, Wq, Wk, Wv, Wo, bo):
    """Host-side shard + layout prep. Returns per-core input maps."""
    Wq = np.asarray(Wq, dtype=np.float32)
    Wk = np.asarray(Wk, dtype=np.float32)
    Wv = np.asarray(Wv, dtype=np.float32)
    Wo = np.asarray(Wo, dtype=np.float32)
    # w8: [64, H, 2, 256] = (Wq | Wk) * 256
    wq_p = np.transpose(Wq.reshape(H, 2, 64, D), (2, 0, 1, 3))
    wk_p = np.transpose(Wk.reshape(H, 2, 64, D), (2, 0, 1, 3))
    w8_p = _e4(np.concatenate([wq_p, wk_p], axis=3), SWQ)
    # wv: [128, H*128]
    wv_p = _bf(np.transpose(Wv, (1, 0, 2)).reshape(D, H * D))
    # wo hi/lo: [128, NPAIR, 2, E] with residual split at scale SWO
    wo_r = np.transpose(Wo.reshape(NPAIR, 2, D, E), (2, 0, 1, 3))
    wo_hi = _e4(wo_r, SWO)
    wo_lo = (wo_r * SWO - wo_hi.astype(np.float32)).astype(E4NP)
    in_maps = []
    for b in range(N_CORES):
        xb = np.asarray(inputs[b], dtype=np.float32)
        # x8: [64, H, 2, S]: x8[p,h,i,s] = 16*x[s, h*128+i*64+p]
        x8_p = _e4(np.transpose(xb.reshape(S, H, 2, 64), (3, 1, 2, 0)), SX)
        # xn packed per head: xn[h][p, c*128+j] = x[c*128+p, h*128+j]
        xn_p = _bf(np.transpose(xb.reshape(NCH, 128, H, D),
                                (2, 1, 0, 3)).reshape(H, 128, NCH * D))
        in_maps.append({"x8": x8_p, "xn": xn_p, "w8": w8_p, "wv": wv_p,
                        "wo_hi": wo_hi, "wo_lo": wo_lo})
    return in_maps


def kernel(inputs, Wq, Wk, Wv, Wo, bo):
    B = inputs.shape[0]
    assert B == N_CORES and inputs.shape[1:] == (S, E)
    nc = get_module()
    in_maps = prepare_in_maps(inputs, Wq, Wk, Wv, Wo, bo)
    res = run_bass_kernel_spmd(nc, in_maps, list(range(N_CORES)))
    outs = np.stack([res.results[b]["out"].astype(np.float32)
                     for b in range(N_CORES)], axis=0)
    return (outs + np.asarray(bo, dtype=np.float32)[None, None, :]).astype(
        np.float32)


# revision 14
# speedup vs baseline: 1.2565x; 1.0155x over previous
"""Trainium2 Bass kernel for nn_MultiHeadLinearAttention.

Full-input contract: kernel(**inputs) takes the unsharded numpy inputs and
returns the full output. Internally: data-parallel over batch across the 8
NeuronCores (B == 8, one batch element per core), no collectives.

Per-core math (S=2048, E=2048, H=16, d=128):
  q/k projections: fp8(e4m3) DoubleRow matmuls, contraction d=128 packed as
    two 64-partition K-tiles (x and W pre-packed [64, 2, *] on host, scales
    x*16, W*256 so the psum holds q*2^12). A tiny fp8 "ones" matmul preloads
    the psum with 2^12 so it holds (q+1)*2^12, which lets
    phi(x) = elu(x)+1 = min(exp(x), max(x+1, 1)) be computed with one Act op
    (Exp with scale 2^-12, bias -1) + one DVE tensor_scalar
    ((in max 4096) * 2^-12) + one DVE min.
  Gram trick (bf16): G[d',d] = sum_s xh[s,d'] pk[s,d]; kv = G^T @ Wv; ksum
    fused as psum col 128.  num/den: bf16; den for all 16 heads of an
    s-chunk lands in one [128,16] psum tile -> single reciprocal.
  ctx: stored as fp8 hi+lo residual pair (scale 2^10), byte-interleaved
    [s, h, d, 2] so ONE uint16-view blocked DMA transpose yields
    [d, h, s, 2] with hi/lo selectable by byte offset.
  Wo stage: fp8 DoubleRow, 3 instructions per head-pair per e-block:
    (c_hi+c_lo) @ w_hi (2 instrs) + c_hi @ w_lo (1 instr) -- 0.75 PE
    cycles per K=128 column vs 1.0 for bf16, with ~bf16-level accuracy
    (Wo pre-split into hi+lo e4m3 on host, scale 2^10).
  Output: psum * 2^-20 -> bf16 SBUF -> DRAM; host upcasts and adds bias.
"""

import numpy as np
import ml_dtypes

import concourse.bass as bass
import concourse.mybir as mybir
import concourse.tile as tile
from concourse import bacc
from concourse.bass_utils import run_bass_kernel_spmd

S = 2048
E = 2048
H = 16
D = 128
N_CORES = 8
NCH = S // 128  # 16 s-chunks
NPAIR = H // 2  # 8 head pairs

F32 = mybir.dt.float32
BF16 = mybir.dt.bfloat16
FP8 = mybir.dt.float8e4
U16 = mybir.dt.uint16
AF = mybir.ActivationFunctionType
ALU = mybir.AluOpType
PM = mybir.MatmulPerfMode

E4NP = ml_dtypes.float8_e4m3
BFNP = ml_dtypes.bfloat16

SX = 16.0       # x -> fp8 scale
SWQ = 256.0     # Wq/Wk -> fp8 scale (psum = q * 2^12)
SC = 1024.0     # ctx -> fp8 scale
SWO = 1024.0    # Wo -> fp8 scale
OUT_DESCALE = 1.0 / (SC * SWO)

_CACHED = {}
DEBUG = False


def build_module():
    nc = bacc.Bacc("TRN2", target_bir_lowering=False, debug=False,
                   num_devices=N_CORES)

    x8 = nc.dram_tensor("x8", [64, H, 2, S], FP8, kind="ExternalInput")
    xn = nc.dram_tensor("xn", [H, 128, NCH * 128], BF16,
                        kind="ExternalInput")
    w8 = nc.dram_tensor("w8", [64, H, 2, 256], FP8, kind="ExternalInput")
    wv = nc.dram_tensor("wv", [128, H * 128], BF16, kind="ExternalInput")
    wo_hi = nc.dram_tensor("wo_hi", [128, NPAIR, 2, E], FP8,
                           kind="ExternalInput")
    wo_lo = nc.dram_tensor("wo_lo", [128, NPAIR, 2, E], FP8,
                           kind="ExternalInput")
    out = nc.dram_tensor("out", [S, E], BF16, kind="ExternalOutput")
    if DEBUG:
        dbg_pqT = nc.dram_tensor("dbg_pqT", [128, H * S], BF16,
                                 kind="ExternalOutput")
        dbg_kv = nc.dram_tensor("dbg_kv", [128, H, 128], BF16,
                                kind="ExternalOutput")
        dbg_ksd = nc.dram_tensor("dbg_ksd", [128, H], BF16,
                                 kind="ExternalOutput")
        dbg_inv = nc.dram_tensor("dbg_inv", [128, 16], F32,
                                 kind="ExternalOutput")
        dbg_chl = nc.dram_tensor("dbg_chl", [128, H, 128, 2], FP8,
                                 kind="ExternalOutput")
        dbg_chlT = nc.dram_tensor("dbg_chlT", [128, H, 128, 2], FP8,
                                  kind="ExternalOutput")
        dbg_g = nc.dram_tensor("dbg_g", [128, 128], BF16,
                               kind="ExternalOutput")

    with tile.TileContext(nc) as tc:
        with (
            tc.tile_pool(name="const", bufs=1) as const,
            tc.tile_pool(name="work", bufs=2) as work,
            tc.tile_pool(name="psum", bufs=2, space="PSUM") as psum,
        ):
            # ---------------- constants / persistent tiles ----------------
            w8_sb = const.tile([64, H, 2, 256], FP8)
            nc.sync.dma_start(out=w8_sb[:], in_=w8[:])
            wv_sb = const.tile([128, H * 128], BF16)
            woh_sb = const.tile([128, NPAIR, 2, E], FP8)
            wol_sb = const.tile([128, NPAIR, 2, E], FP8)

            pre_l = const.tile([1, 2, 128], FP8)
            nc.vector.memset(pre_l[:], 32.0)
            pre_r = const.tile([1, 2, 512], FP8)
            nc.vector.memset(pre_r[:], 64.0)
            neg1 = const.tile([128, 1], F32)
            nc.vector.memset(neg1[:], -1.0)
            ones = const.tile([128, 1], BF16)
            nc.vector.memset(ones[:], 1.0)

            pqT = const.tile([128, H * S], BF16)       # all heads, transposed
            kv_all = const.tile([128, H, 128], BF16)   # kv per head
            ksd = const.tile([128, H], BF16)           # ksum * 2^-10 per head

            # ---------------- helpers ----------------
            def phi(src_psum, dst, n):
                """psum holds (v+1)*2^12; writes phi(v) [128, n] bf16."""
                e = work.tile([128, 512], BF16, tag="e", bufs=2)
                nc.scalar.activation(e[:, 0:n], src_psum[:, 0:n], AF.Exp,
                                     bias=neg1[:, 0:1], scale=2.0 ** -12)
                t = work.tile([128, 512], BF16, tag="t", bufs=2)
                nc.vector.tensor_scalar(t[:, 0:n], src_psum[:, 0:n], 4096.0,
                                        2.0 ** -12, ALU.max, ALU.mult)
                nc.vector.tensor_tensor(dst, e[:, 0:n], t[:, 0:n], ALU.min)

            def preload(p, n):
                nc.tensor.matmul(p[:, 0:n], pre_l[:], pre_r[:, :, 0:n],
                                 start=True, stop=False,
                                 perf_mode=PM.DoubleRow)

            # deferred q quarters: (h, quarter) for quarters 1..3
            q_queue = [(h, qt) for qt in (1, 2, 3) for h in range(H)]
            q_queue.reverse()
            q_pf = list(q_queue)
            q_pending = []

            def prefetch_q_quarter():
                h, qt = q_pf.pop()
                xq = work.tile([64, 2, 512], FP8, tag="xq", bufs=5)
                nc.sync.dma_start(
                    out=xq[:], in_=x8[:, h, :, qt * 512:(qt + 1) * 512])
                q_pending.append(xq)

            def emit_q_quarter(h, qt):
                xq = q_pending.pop(0)
                qp = psum.tile([128, 512], F32, tag="qk", bufs=2)
                preload(qp, 512)
                nc.tensor.matmul(
                    qp[:], w8_sb[:, h, :, 0:128], xq[:],
                    start=False, stop=True, perf_mode=PM.DoubleRow)
                phi(qp, pqT[:, h * S + qt * 512:h * S + (qt + 1) * 512], 512)

            def drain_q(k):
                for _ in range(k):
                    while q_pf and len(q_pending) < 4:
                        prefetch_q_quarter()
                    if q_queue:
                        emit_q_quarter(*q_queue.pop())

            # ---------------- pass A: per head ----------------
            for h in range(H):
                x8_h = work.tile([64, 2, S], FP8, tag="x8h", bufs=2)
                nc.sync.dma_start(out=x8_h[:], in_=x8[:, h])
                if h == 0:
                    nc.sync.dma_start(out=wv_sb[:], in_=wv[:])
                if h == 1:
                    nc.sync.dma_start(out=woh_sb[:], in_=wo_hi[:])
                if h == 2:
                    nc.sync.dma_start(out=wol_sb[:], in_=wo_lo[:])
                xn_h = work.tile([128, NCH, 128], BF16, tag="xn", bufs=2)
                nc.sync.dma_start(
                    out=xn_h[:], in_=xn[h].rearrange("p (c j) -> p c j",
                                                     j=128))

                # k projection (natural layout) + phi -> pk
                pk = work.tile([128, NCH, 128], BF16, tag="pk", bufs=2)
                for g in range(4):
                    kp = psum.tile([128, 512], F32, tag="qk", bufs=2)
                    preload(kp, 512)
                    for c in range(4):
                        ch = g * 4 + c
                        nc.tensor.matmul(
                            kp[:, c * 128:(c + 1) * 128],
                            x8_h[:, :, ch * 128:(ch + 1) * 128],
                            w8_sb[:, h, :, 128:256],
                            start=False, stop=(c == 3),
                            perf_mode=PM.DoubleRow)
                    phi(kp, pk[:, 4 * g:4 * g + 4, :].rearrange(
                        "p c j -> p (c j)"), 512)

                # q projection quarter 0 (transposed layout)
                qp = psum.tile([128, 512], F32, tag="qk", bufs=2)
                preload(qp, 512)
                nc.tensor.matmul(qp[:], w8_sb[:, h, :, 0:128],
                                 x8_h[:, :, 0:512], start=False, stop=True,
                                 perf_mode=PM.DoubleRow)
                phi(qp, pqT[:, h * S:h * S + 512], 512)

                # Gram G + ksum (bf16)
                gp_t = psum.tile([128, 128], F32, tag="g", bufs=1)
                gp = gp_t[:]
                kvp_t = psum.tile([128, 129], F32, tag="kv", bufs=1)
                kvp = kvp_t[:]
                for c in range(NCH):
                    nc.tensor.matmul(gp, xn_h[:, c, :], pk[:, c, :],
                                     start=(c == 0), stop=(c == NCH - 1),
                                     skip_group_check=True)
                    nc.tensor.matmul(kvp[:, 128:129], pk[:, c, :], ones[:],
                                     start=(c == 0), stop=(c == NCH - 1),
                                     skip_group_check=True)
                g_sb = work.tile([128, 128], BF16, tag="gsb", bufs=2)
                nc.scalar.activation(g_sb[:], gp, AF.Copy)
                if DEBUG and h == 0:
                    nc.sync.dma_start(out=dbg_g[:], in_=g_sb[:])
                nc.tensor.matmul(kvp[:, 0:128], g_sb[:],
                                 wv_sb[:, h * 128:(h + 1) * 128],
                                 start=True, stop=True, skip_group_check=True)
                nc.scalar.activation(kv_all[:, h, :], kvp[:, 0:128], AF.Copy)
                nc.scalar.activation(ksd[:, h:h + 1], kvp[:, 128:129],
                                     AF.Copy, scale=2.0 ** -10)

            if DEBUG:
                nc.sync.dma_start(out=dbg_pqT[:, 0:512], in_=pqT[:, 0:512])
                nc.sync.dma_start(out=dbg_kv[:], in_=kv_all[:])
                nc.sync.dma_start(out=dbg_ksd[:], in_=ksd[:])
            # ---------------- pass B: per s-chunk ----------------
            chlT_tiles = {}

            def emit_wo(wsc):
                chlT_w = chlT_tiles.pop(wsc)
                for eb in range(4):
                    wop = psum.tile([128, 512], F32, tag="wo", bufs=2)
                    esl = slice(eb * 512, (eb + 1) * 512)
                    for j in range(NPAIR):
                        hsl = slice(2 * j, 2 * j + 2)
                        nc.tensor.matmul(
                            wop[:], chlT_w[:, hsl, :, 0],
                            woh_sb[:, j, :, esl], start=(j == 0), stop=False,
                            perf_mode=PM.DoubleRow)
                        nc.tensor.matmul(
                            wop[:], chlT_w[:, hsl, :, 1],
                            woh_sb[:, j, :, esl], start=False, stop=False,
                            perf_mode=PM.DoubleRow)
                        nc.tensor.matmul(
                            wop[:], chlT_w[:, hsl, :, 0],
                            wol_sb[:, j, :, esl], start=False,
                            stop=(j == NPAIR - 1), perf_mode=PM.DoubleRow)
                    out_sb = work.tile([128, 512], BF16, tag="osb", bufs=2)
                    nc.scalar.activation(out_sb[:], wop[:], AF.Copy,
                                         scale=OUT_DESCALE)
                    nc.sync.dma_start(
                        out=out[wsc * 128:(wsc + 1) * 128, esl], in_=out_sb[:])
                    drain_q(1)

            for sc in range(NCH):
                # den for all heads -> one [128, 16] psum; then reciprocal
                den_t = psum.tile([128, 129], F32, tag="kv", bufs=1)
                denp = den_t[:, 0:16]
                for h in range(H):
                    nc.tensor.matmul(
                        denp[:, h:h + 1],
                        pqT[:, h * S + sc * 128:h * S + (sc + 1) * 128],
                        ksd[:, h:h + 1], start=True, stop=True,
                        skip_group_check=True)
                invsc = work.tile([128, 16], F32, tag="inv", bufs=2)
                nc.vector.reciprocal(invsc[:], denp)
                if DEBUG and sc == 0:
                    nc.sync.dma_start(out=dbg_inv[:], in_=invsc[:])  # = 1024/den

                drain_q(1)

                # num + ctx scale to bf16 per head (DVE/Act alternating),
                # then one batched fp8 hi-cast + one batched lo-subtract
                chl = work.tile([128, H, 128, 2], FP8, tag="chl", bufs=2)
                cbf = work.tile([128, H, 128], BF16, tag="cbf", bufs=1)
                for h in range(H):
                    ndp = psum.tile([128, 128], F32, tag="nd", bufs=2)
                    nc.tensor.matmul(
                        ndp[:],
                        pqT[:, h * S + sc * 128:h * S + (sc + 1) * 128],
                        kv_all[:, h, :], start=True, stop=True)
                    if h % 2 == 0:
                        nc.vector.tensor_scalar(cbf[:, h, :], ndp[:],
                                                invsc[:, h:h + 1], None,
                                                ALU.mult)
                    else:
                        nc.scalar.activation(cbf[:, h, :], ndp[:], AF.Copy,
                                             scale=invsc[:, h:h + 1])
                    if h in (5, 11):
                        drain_q(1)
                nc.scalar.activation(chl[:, :, :, 0], cbf[:], AF.Copy)
                nc.vector.tensor_tensor(chl[:, :, :, 1], cbf[:],
                                        chl[:, :, :, 0], ALU.subtract)

                # one blocked u16 transpose: [s,(h d)] -> [d, h, s] pairs
                if DEBUG and sc == 0:
                    nc.sync.dma_start(out=dbg_chl[:], in_=chl[:])
                chlT = work.tile([128, H, 128, 2], FP8, tag="chlT", bufs=3)
                nc.scalar.dma_start(
                    out=chlT[:].rearrange("p h s two -> p h (s two)").bitcast(
                        U16),
                    in_=chl[:].rearrange("p h d two -> p (h d two)").bitcast(
                        U16),
                    transpose=True)

                if DEBUG and sc == 0:
                    nc.sync.dma_start(out=dbg_chlT[:], in_=chlT[:])
                chlT_tiles[sc] = chlT
                if sc > 0:
                    emit_wo(sc - 1)
                if sc == NCH - 1:
                    emit_wo(sc)

            if DEBUG:
                nc.sync.dma_start(out=dbg_pqT[:], in_=pqT[:])
    nc.compile()
    return nc


def get_module():
    if "nc" not in _CACHED:
        _CACHED["nc"] = build_module()
    return _CACHED["nc"]


def _e4(a, scale):
    return (np.ascontiguousarray(a, dtype=np.float32) * scale).astype(E4NP)


def _bf(a):
    return np.ascontiguousarray(a, dtype=np.float32).astype(BFNP)


def prepare_in_maps(inputs, Wq, Wk, Wv, Wo, bo):
    """Host-side shard + layout prep. Returns per-core input maps."""
    Wq = np.asarray(Wq, dtype=np.float32)
    Wk = np.asarray(Wk, dtype=np.float32)
    Wv = np.asarray(Wv, dtype=np.float32)
    Wo = np.asarray(Wo, dtype=np.float32)
    # w8: [64, H, 2, 256] = (Wq | Wk) * 256
    wq_p = np.transpose(Wq.reshape(H, 2, 64, D), (2, 0, 1, 3))
    wk_p = np.transpose(Wk.reshape(H, 2, 64, D), (2, 0, 1, 3))
    w8_p = _e4(np.concatenate([wq_p, wk_p], axis=3), SWQ)
    # wv: [128, H*128]
    wv_p = _bf(np.transpose(Wv, (1, 0, 2)).reshape(D, H * D))
    # wo hi/lo: [128, NPAIR, 2, E] with residual split at scale SWO
    wo_r = np.transpose(Wo.reshape(NPAIR, 2, D, E), (2, 0, 1, 3))
    wo_hi = _e4(wo_r, SWO)
    wo_lo = (wo_r * SWO - wo_hi.astype(np.float32)).astype(E4NP)
    in_maps = []
    for b in range(N_CORES):
        xb = np.asarray(inputs[b], dtype=np.float32)
        # x8: [64, H, 2, S]: x8[p,h,i,s] = 16*x[s, h*128+i*64+p]
        x8_p = _e4(np.transpose(xb.reshape(S, H, 2, 64), (3, 1, 2, 0)), SX)
        # xn packed per head: xn[h][p, c*128+j] = x[c*128+p, h*128+j]
        xn_p = _bf(np.transpose(xb.reshape(NCH, 128, H, D),
                                (2, 1, 0, 3)).reshape(H, 128, NCH * D))
        in_maps.append({"x8": x8_p, "xn": xn_p, "w8": w8_p, "wv": wv_p,
                        "wo_hi": wo_hi, "wo_lo": wo_lo})
    return in_maps


def kernel(inputs, Wq, Wk, Wv, Wo, bo):
    B = inputs.shape[0]
    assert B == N_CORES and inputs.shape[1:] == (S, E)
    nc = get_module()
    in_maps = prepare_in_maps(inputs, Wq, Wk, Wv, Wo, bo)
    res = run_bass_kernel_spmd(nc, in_maps, list(range(N_CORES)))
    outs = np.stack([res.results[b]["out"].astype(np.float32)
                     for b in range(N_CORES)], axis=0)
    return (outs + np.asarray(bo, dtype=np.float32)[None, None, :]).astype(
        np.float32)
